# revision 48
# baseline (speedup 1.0000x reference)
"""Causal self-attention kernel for 8 trn2 NeuronCores.

Sharding: 2 batch groups x 4 tensor-parallel ranks (Megatron-style head
split).  Core c handles batch b=c//4 and heads [4r, 4r+4) with r=c%4.
All matmul operands are bf16 (f32 PSUM accumulation); host casts inputs.
Each core:
  1. qk^T projection:   qkT[128h:(128h+128), :] = [q_h^T; k_h^T]  (64+64 rows)
  2. v projection:      v[token, 65h:65h+64], col 65h+64 = 1.0 (den trick)
  3. causal attention in s^T = [key_part, query_free] layout:
       sT = (k^T slice) matmul q^T ; p = exp(s/8) * mask ; y'T += [v|1].T p
     row 64 of y'T is the softmax denominator; normalize via scalar-engine
     reciprocal + DMA partition-broadcast + multiply.
  4. partial out = y_own @ w_out[own head rows, :]  -> [2048, 1024]
  5. ReduceScatter(add) across the 4-rank group straight into the bf16
     output: rank r keeps the summed rows [512r, 512r+512) of each chunk.
Host concatenates the 8 x [512, 1024] outputs into [2, 2048, 1024] (f32).
"""

import sys

for _p in ("/opt/trn_rl_repo", "/root/.axon_site", "/root/.axon_site/_ro/trn_rl_repo",
           "/root/.axon_site/_ro/pypackages"):
    if _p not in sys.path:
        sys.path.append(_p)

import ml_dtypes
import numpy as np

import concourse.mybir as mybir
import concourse.tile as tile
from concourse import bacc
from concourse import bass_utils

F32 = mybir.dt.float32
BF16 = mybir.dt.bfloat16
NP_BF16 = ml_dtypes.bfloat16


def _cfg(B=2, T=2048, C=1024, H=16, n_cores=8, tp=4):
    D = 64
    assert C == H * D
    cfg = dict(B=B, T=T, C=C, H=H, D=D, n_cores=n_cores, tp=tp)
    cfg["groups"] = [[g * tp + r for r in range(tp)] for g in range(n_cores // tp)]
    cfg["HPC"] = H // tp           # heads per core
    cfg["KT"] = C // 128           # contraction tiles for projections
    cfg["NQ"] = T // 512           # 512-wide query chunks
    cfg["TT"] = T // 128           # 128-wide token (key) tiles
    cfg["RT"] = T // tp            # output rows per core
    assert cfg["RT"] % 128 == 0 and T % 512 == 0
    return cfg


CFG = _cfg()


def build_nc(cfg=CFG, reps=1, no_rs=False):
    B, T, C, H, D = cfg["B"], cfg["T"], cfg["C"], cfg["H"], cfg["D"]
    HPC, KT, NQ, TT, RT = cfg["HPC"], cfg["KT"], cfg["NQ"], cfg["TT"], cfg["RT"]
    tp = cfg["tp"]
    assert HPC % 2 == 0
    Exp = mybir.ActivationFunctionType.Exp
    dt_mm = BF16

    nc = bacc.Bacc("TRN2", target_bir_lowering=False, debug=False,
                   enable_asserts=True, num_devices=cfg["n_cores"])

    # host-packed layouts: one DMA per logical tensor / chunk
    xTp = nc.dram_tensor("xTp", [128, NQ * KT * 512], dt_mm, kind="ExternalInput")
    w_qk = nc.dram_tensor("w_qk", [128, KT * HPC * 128], dt_mm, kind="ExternalInput")
    w_v = nc.dram_tensor("w_v", [128, KT * HPC * 64], dt_mm, kind="ExternalInput")
    w_out = nc.dram_tensor("w_out", [128, 2 * C], dt_mm, kind="ExternalInput")
    b_bcast = nc.dram_tensor("b_bcast", [128, C], F32, kind="ExternalInput")
    mask = nc.dram_tensor("mask", [128, 128], dt_mm, kind="ExternalInput")
    ones = nc.dram_tensor("ones", [128, 64], dt_mm, kind="ExternalInput")
    out = nc.dram_tensor("out", [NQ * (512 // tp), C], BF16, kind="ExternalOutput")

    def mm(o, lhsT, rhs, **kw):
        nc.tensor.matmul(o, lhsT, rhs, **kw)

    n_yt = (HPC * 64 + 127) // 128   # SBUF tiles holding this core's y^T
    rw = 512 // tp

    with tile.TileContext(nc) as tc:
        with (
            tc.tile_pool(name="persist", bufs=1) as per_pool,
            tc.tile_pool(name="xt", bufs=2) as xt_pool,
            tc.tile_pool(name="pT", bufs=4) as pT_pool,
            tc.tile_pool(name="norm", bufs=3) as norm_pool,
            tc.tile_pool(name="osb", bufs=4) as o_pool,
            tc.tile_pool(name="ps_s", bufs=2, space="PSUM") as ps_s,
            tc.tile_pool(name="ps_y", bufs=2, space="PSUM") as ps_y,
            tc.tile_pool(name="ps_acc", bufs=2, space="PSUM") as ps_acc,
            tc.tile_pool(name="dram", bufs=1, space="DRAM") as dram_pool,
        ):
          for _rep in range(reps):
            # wqk k=0 slice on the scalar queue so it transfers in
            # parallel with the x^T k=0 slice on the sync queue -- both
            # gate the very first matmul
            wqk_sb = per_pool.tile([128, KT * HPC * 128], dt_mm, name="wqk", tag="wqk")
            nc.scalar.dma_start(wqk_sb[:, 0:512], w_qk[:, 0:512])
            qkT_sb = [per_pool.tile([128, 2 * T], dt_mm, name=f"qkT{hp}", tag=f"qkT{hp}")
                      for hp in range(HPC // 2)]
            v_sb = [per_pool.tile([128, HPC * 65], dt_mm, name=f"v{mt}", tag=f"v{mt}")
                    for mt in range(TT)]
            yT_sb = [per_pool.tile([128, T], dt_mm, name=f"yT{i}", tag=f"yT{i}")
                     for i in range(n_yt)]
            rs_in = [[dram_pool.tile([512, 512], BF16, name=f"rsi{qc}_{h}",
                                     tag=f"rsi{qc}_{h}") for h in range(2)]
                     for qc in range(NQ)]
            # one tile for all RS outputs: the single final copy then
            # depends on every RS, so the scheduler places it dead last
            # and no queue ever blocks on a collective mid-kernel
            rs_out_all = dram_pool.tile([NQ * 2 * rw, 512], BF16,
                                        name="rso", tag="rso")
            rs_out = [[rs_out_all[rw * (h * NQ + qc):rw * (h * NQ + qc + 1), :]
                       for h in range(2)] for qc in range(NQ)]
            # skew-absorbing dummy collectives: the first RS after a
            # cross-core drift period absorbs the skew in its rendezvous
            # (~16 GB/s vs ~43 GB/s warm).  tiny dummies, anchored by a
            # real data dependency (so the scheduler cannot float them to
            # the program start), absorb the skew during compute instead.
            # each has private in/out tiles: a shared out tile creates a
            # WAR dep that stalls the gpsimd queue until the prior dummy
            # completes.
            sync_io = [(dram_pool.tile([4, 64], F32, name=f"sy_i{i}",
                                       tag=f"sy_i{i}"),
                        dram_pool.tile([1, 64], F32, name=f"sy_o{i}",
                                       tag=f"sy_o{i}")) for i in range(3)]

            def absorb_skew(i, anchor_sb):
                s_in, s_out = sync_io[i]
                nc.sync.dma_start(s_in[:], anchor_sb)
                nc.gpsimd.collective_compute(
                    "ReduceScatter", mybir.AluOpType.add,
                    replica_groups=cfg["groups"],
                    ins=[s_in[:].opt()], outs=[s_out[:].opt()])


            # schedule: att(n) right after proj(n), so each chunk's
            # ReduceScatter is issued early and the CC core is never
            # backlogged at the end of the kernel.
            steps = []
            for n in range(NQ):
                steps += [("proj", n), ("att", n)]
            # xt tiles created up front so attention steps can prefetch
            # the next chunk's x^T while the PE is busy.
            xt_tiles = {}

            def load_xt(n):
                xt_all = xt_pool.tile([128, KT * 512], dt_mm, name="xt", tag="xt")
                base = KT * 512 * n
                nc.sync.dma_start(xt_all[:], xTp[:, base:base + KT * 512])
                xt_tiles[n] = xt_all

            pending_outproj = []

            def emit_outproj(qc):
                # column-half-major so each half's ReduceScatter starts as
                # soon as its four 128-row blocks are written
                for nn in range(C // 512):
                    for j in range(4):
                        m = 4 * qc + j
                        # alternate PSUM pools: 4 rotating accumulators so
                        # the matmul chain never waits on the bias-add
                        # eviction (the "s" slots are idle here)
                        if j % 2 == 0:
                            acc_t = ps_acc.tile([128, 512], F32, name="acc",
                                                tag="acc")
                            acc = acc_t[:]
                        else:
                            acc_t = ps_s.tile([128, 1024], F32, name="s", tag="s")
                            acc = acc_t[:, 0:512]
                        for k in range(n_yt):
                            mm(acc, yT_sb[k][:, 128 * m:128 * (m + 1)],
                               wout_sb[:, C * k + 512 * nn:C * k + 512 * (nn + 1)],
                               start=(k == 0), stop=(k == n_yt - 1))
                        po_sb = o_pool.tile([128, 512], BF16, name="po", tag="po")
                        nc.vector.tensor_add(po_sb[:], acc,
                                             bb_sb[:, 512 * nn:512 * (nn + 1)])
                        nc.sync.dma_start(
                            rs_in[qc][nn][128 * j:128 * (j + 1), :], po_sb[:])
                    if no_rs:
                        nc.sync.dma_start(rs_out[qc][nn], rs_in[qc][nn][0:rw, :])
                    else:
                        nc.gpsimd.collective_compute(
                            "ReduceScatter", mybir.AluOpType.add,
                            replica_groups=cfg["groups"],
                            ins=[rs_in[qc][nn][:].opt()],
                            outs=[rs_out[qc][nn].opt()])

            for kind, n in steps:
              if kind == "proj":
                # ---- x^T chunk load + qk/v projections ---------------
                if n == 0:
                    # split loads so the first accumulation chain starts
                    # after ~0.25 MB instead of the full 2.5 MB
                    xt_all = xt_pool.tile([128, KT * 512], dt_mm, name="xt", tag="xt")
                    nc.sync.dma_start(xt_all[:, 0:512], xTp[:, 0:512])
                    nc.sync.dma_start(wqk_sb[:, 512:], w_qk[:, 512:])
                    nc.sync.dma_start(xt_all[:, 512:], xTp[:, 512:KT * 512])
                    xt_tiles[0] = xt_all
                    wv_sb = per_pool.tile([128, KT * HPC * 64], dt_mm,
                                          name="wv", tag="wv")
                    nc.sync.dma_start(wv_sb[:], w_v[:, :])
                    ones_sb = per_pool.tile([128, 64], dt_mm, name="ones", tag="ones")
                    nc.sync.dma_start(ones_sb[:], ones[:, :])
                elif n not in xt_tiles:
                    load_xt(n)
                xt_all = xt_tiles[n]
                xt = lambda k: xt_all[:, 512 * k:512 * (k + 1)]
                for m in range(HPC):
                    hp, is_k = divmod(m, 2)
                    acc = ps_acc.tile([128, 512], F32, name="acc", tag="acc")
                    for k in range(KT):
                        mm(acc[:], wqk_sb[:, 512 * k + 128 * m:512 * k + 128 * (m + 1)],
                           xt(k), start=(k == 0), stop=(k == KT - 1))
                    off = (T if is_k else 0) + 512 * n
                    # alternate eviction engines so psum slots free faster
                    if m % 2 == 0:
                        nc.scalar.copy(qkT_sb[hp][:, off:off + 512], acc[:])
                    else:
                        nc.vector.tensor_copy(qkT_sb[hp][:, off:off + 512], acc[:])
                for j in range(4):
                    mt = 4 * n + j
                    acc = ps_acc.tile([128, HPC * 64], F32, name="acc", tag="acc")
                    for k in range(KT):
                        mm(acc[:], xt(k)[:, 128 * j:128 * (j + 1)],
                           wv_sb[:, HPC * 64 * k:HPC * 64 * (k + 1)],
                           start=(k == 0), stop=(k == KT - 1))
                    vt = v_sb[mt]
                    vsrc = acc[:].rearrange("p (h e) -> p h e", e=64)
                    vdst = vt[:].rearrange("p (h e) -> p h e", e=65)[:, :, 0:64]
                    nc.vector.tensor_copy(vdst, vsrc)
                    nc.vector.tensor_copy(
                        vt[:].rearrange("p (h e) -> p h e", e=65)[:, :, 64:65],
                        ones_sb[:, 0:HPC].rearrange("p (h e) -> p h e", e=1))
                if n == 0:
                    msk_sb = per_pool.tile([128, 128], dt_mm, name="mask", tag="mask")
                    nc.sync.dma_start(msk_sb[:], mask[:, :])
                    bb_sb = per_pool.tile([128, C], F32, name="bb", tag="bb")
                    nc.sync.dma_start(bb_sb[:], b_bcast[:, :])
                    wout_sb = per_pool.tile([128, 2 * C], dt_mm, name="wout",
                                            tag="wout")
                    nc.sync.dma_start(wout_sb[:], w_out[:, :])
                if pending_outproj:
                    emit_outproj(pending_outproj.pop(0))

                continue
              else:
                # ---- attention for query chunk qc = n ----------------
                qc = n
                # prefetch the next proj chunk's x^T while the PE is busy
                # with attention (sync queue is idle at this point)
                for kk, nn2 in steps[steps.index((kind, n)) + 1:]:
                    if kk == "proj" and nn2 not in xt_tiles:
                        load_xt(nn2)
                        break
                for h in range(HPC):
                    hp, half_h = divmod(h, 2)
                    base = 64 * half_h
                    qT = qkT_sb[hp][base:base + 64, 0:T]
                    kT = qkT_sb[hp][base:base + 64, T:2 * T]
                    y_acc = ps_y.tile([65, 512], F32, name="y", tag="y")

                    # software-pipelined units: the S matmuls + exp of unit
                    # u+1 are emitted before the PV matmuls of unit u, so
                    # the PE never waits for the scalar-engine exp.
                    units = [("pair", kt) for kt in range(0, 4 * qc, 2)]
                    units += [("diag", i) for i in range(4)]

                    def emit_s(u):
                        kind_u, a = u
                        s_ps = ps_s.tile([128, 1024], F32, name="s", tag="s")
                        pT = pT_pool.tile([128, 1024], dt_mm, name="p", tag="p")
                        if kind_u == "pair":
                            for half_i in range(2):
                                mm(s_ps[:, 512 * half_i:512 * (half_i + 1)],
                                   kT[:, 128 * (a + half_i):128 * (a + half_i + 1)],
                                   qT[:, 512 * qc:512 * (qc + 1)],
                                   start=True, stop=True)
                            nc.scalar.activation(pT[:], s_ps[:], Exp, scale=0.125)
                        else:
                            lo = 128 * a
                            mm(s_ps[:, lo:512], kT[:, 128 * (4 * qc + a):
                                                   128 * (4 * qc + a + 1)],
                               qT[:, 512 * qc + lo:512 * (qc + 1)],
                               start=True, stop=True)
                            nc.scalar.activation(pT[:, lo:512], s_ps[:, lo:512],
                                                 Exp, scale=0.125)
                            nc.vector.tensor_mul(
                                pT[:, lo:lo + 128], pT[:, lo:lo + 128], msk_sb[:])
                        return (kind_u, a, pT)

                    def emit_pv(state, first, last):
                        kind_u, a, pT = state
                        if kind_u == "pair":
                            for half_i in range(2):
                                mm(y_acc[:], v_sb[a + half_i][:, 65 * h:65 * h + 65],
                                   pT[:, 512 * half_i:512 * (half_i + 1)],
                                   start=first, stop=False)
                                first = False
                        else:
                            lo = 128 * a
                            mm(y_acc[:, lo:512],
                               v_sb[4 * qc + a][:, 65 * h:65 * h + 65],
                               pT[:, lo:512], start=first, stop=last)

                    prev = None
                    for ui, u in enumerate(units):
                        cur = emit_s(u)
                        if prev is not None:
                            emit_pv(prev, first=(ui == 1), last=False)
                        prev = cur
                    emit_pv(prev, first=(len(units) == 1), last=True)

                    # normalize: row 64 of y_acc is the denominator.  copy
                    # it to partition 0 first (the custom-DVE reciprocal
                    # drops the input partition offset), reciprocal, then
                    # partition-broadcast on the idle gpsimd engine.
                    d_sb = norm_pool.tile([1, 512], F32, name="d", tag="d")
                    nc.vector.tensor_copy(d_sb[:], y_acc[64:65, :])
                    r_sb = norm_pool.tile([1, 512], F32, name="r", tag="r")
                    nc.vector.reciprocal_approx_fast(r_sb[:], d_sb[:])
                    rb_sb = norm_pool.tile([64, 512], F32, name="rb", tag="rb")
                    nc.gpsimd.partition_broadcast(rb_sb[:], r_sb[:])
                    ti, po = divmod(64 * h, 128)
                    nc.vector.tensor_mul(
                        yT_sb[ti][po:po + 64, 512 * qc:512 * (qc + 1)],
                        y_acc[0:64, :], rb_sb[:])
                    # second skew absorber: anchored at the LAST chunk's
                    # first head, so its rendezvous hides under the
                    # remaining three heads (~27us) and the critical-path
                    # final RS pair then runs at the warm rate
                    if not no_rs and qc == NQ - 1 and h == 0:
                        absorb_skew(1, rb_sb[0:4, 0:64])

                # first skew absorber after att 0 (absorbs startup-barrier
                # exit skew; an absorber with too little compute behind it
                # just delays the next real RS instead)
                if not no_rs and qc == 0:
                    absorb_skew(0, rb_sb[0:4, 0:64])
                # out-proj for the last chunk runs now; earlier chunks'
                # out-proj is deferred into the next proj step so the PE
                # has ~10us of projection work to cover the normalize
                # latency of the last head (see emit_outproj in proj).
                if qc == NQ - 1:
                    emit_outproj(qc)
                else:
                    pending_outproj.append(qc)
            # keep-alive matmuls: the final RS pair runs ~2x slower once
            # the PE goes idle (chip-level DVFS clocks the links down with
            # it).  dead matmuls anchored on the last chunk's yT (so the
            # scheduler keeps them at the end) run concurrently with the
            # final collectives and keep the power state up.  they are
            # shorter than the RS tail either way, so worst case they are
            # hidden behind it.
            if not no_rs:
                anchor = yT_sb[0][:, 512 * (NQ - 1):512 * NQ]
                for i in range(80):
                    if i % 2 == 0:
                        ka_t = ps_acc.tile([128, 512], F32, name="acc",
                                           tag="acc")
                        ka = ka_t[:]
                    else:
                        ka_t = ps_s.tile([128, 1024], F32, name="s", tag="s")
                        ka = ka_t[:, 0:512]
                    mm(ka, wout_sb[:, 0:128], anchor, start=True, stop=True)
            # two final copies (one per column half), each depending on
            # the four RS ops of that half -- scheduled after the last RS
            for h in range(2):
                nc.sync.dma_start(
                    out[:, 512 * h:512 * (h + 1)],
                    rs_out_all[rw * h * NQ:rw * (h + 1) * NQ, :])
    nc.compile()
    return nc


def shard_inputs(x, w_qkv, w_out, b_out, cfg=CFG):
    B, T, C, H, D, tp = (cfg["B"], cfg["T"], cfg["C"], cfg["H"], cfg["D"], cfg["tp"])
    HPC, KT, NQ = cfg["HPC"], cfg["KT"], cfg["NQ"]
    x = np.asarray(x, dtype=np.float32)
    w_qkv = np.asarray(w_qkv, dtype=np.float32)
    w_out = np.asarray(w_out, dtype=np.float32)
    b_out = np.asarray(b_out, dtype=np.float32)

    w_q, w_k, w_v = w_qkv[:, :C], w_qkv[:, C:2 * C], w_qkv[:, 2 * C:]
    kp = np.arange(128)[:, None]
    qf = np.arange(128)[None, :]
    mask = (kp <= qf).astype(NP_BF16)
    b_bcast = np.ascontiguousarray(np.broadcast_to(b_out / tp, (128, C)))
    ones_np = np.ones((128, 64), dtype=NP_BF16)

    in_maps = []
    for c in range(cfg["n_cores"]):
        b, r = divmod(c, tp)
        heads = list(range(HPC * r, HPC * (r + 1)))
        blocks = []
        for hp in range(len(heads) // 2):
            g0, g1 = heads[2 * hp], heads[2 * hp + 1]
            blocks.append(np.concatenate(
                [w_q[:, 64 * g0:64 * (g0 + 1)], w_q[:, 64 * g1:64 * (g1 + 1)]], axis=1))
            blocks.append(np.concatenate(
                [w_k[:, 64 * g0:64 * (g0 + 1)], w_k[:, 64 * g1:64 * (g1 + 1)]], axis=1))
        wqk_c = np.concatenate(blocks, axis=1)          # [C, HPC*128]
        wv_c = np.concatenate([w_v[:, 64 * g:64 * (g + 1)] for g in heads], axis=1)
        wout_c = np.concatenate([w_out[64 * g:64 * (g + 1), :] for g in heads], axis=0)

        # pack for single-DMA loads (partition dim 128 first):
        # xTp[:, n*KT*512 + k*512 + t] = x[b].T[128k+p, 512n+t]
        xT = np.ascontiguousarray(x[b].T).astype(NP_BF16)          # [C, T]
        xTp = (xT.reshape(KT, 128, NQ, 512).transpose(1, 2, 0, 3)
               .reshape(128, NQ * KT * 512))
        # w_qk packed: [:, k*HPC*128 + f] = wqk_c[128k+p, f]
        wqk_p = (wqk_c.astype(NP_BF16).reshape(KT, 128, HPC * 128)
                 .transpose(1, 0, 2).reshape(128, KT * HPC * 128))
        wv_p = (wv_c.astype(NP_BF16).reshape(KT, 128, HPC * 64)
                .transpose(1, 0, 2).reshape(128, KT * HPC * 64))
        wout_p = (wout_c.astype(NP_BF16).reshape(2, 128, C)
                  .transpose(1, 0, 2).reshape(128, 2 * C))
        in_maps.append({
            "xTp": np.ascontiguousarray(xTp),
            "w_qk": np.ascontiguousarray(wqk_p),
            "w_v": np.ascontiguousarray(wv_p),
            "w_out": np.ascontiguousarray(wout_p),
            "b_bcast": b_bcast,
            "mask": mask,
            "ones": ones_np,
        })
    return in_maps


def assemble(results, cfg=CFG):
    B, T, C, tp, NQ = cfg["B"], cfg["T"], cfg["C"], cfg["tp"], cfg["NQ"]
    rw = 512 // tp
    out = np.empty((B, T, C), dtype=np.float32)
    for c in range(cfg["n_cores"]):
        b, r = divmod(c, tp)
        o = np.asarray(results[c]["out"], dtype=np.float32)
        for qc in range(NQ):
            out[b, 512 * qc + rw * r:512 * qc + rw * (r + 1), :] = \
                o[rw * qc:rw * (qc + 1)]
    return out


_NC_CACHE = {}


def _get_nc(cfg_key="default", cfg=CFG):
    if cfg_key not in _NC_CACHE:
        _NC_CACHE[cfg_key] = build_nc(cfg)
    return _NC_CACHE[cfg_key]


def kernel(x, w_qkv, w_out, b_out):
    cfg = CFG
    nc = _get_nc()
    in_maps = shard_inputs(x, w_qkv, w_out, b_out, cfg)
    res = bass_utils.run_bass_kernel_spmd(
        nc, in_maps, core_ids=list(range(cfg["n_cores"])))
    return assemble(res.results, cfg)


if __name__ == "__main__":
    print("module loads ok")
